# revision 34
# baseline (speedup 1.0000x reference)
"""K-means step kernel for Trainium2 (8 NeuronCores, data-parallel over n).

scores[n,k] = ||c_k||^2 - 2 x_n.c_k ; assign = argmin_k ; new centroids =
segment-mean.  Strategy per core (n_loc = n/8 rows, 128 tiles of 128 rows):

  mm1 (PE):  psum = x16 @ (-2C)16^T in one fp16 pass (fp32 PSUM accumulate)
             with csq added IN PSUM: each 512-col group gets one extra
             fp16 contraction-2 matmul ones[2,128]^T @ [csq_hi; csq_lo]
             (2-term fp16 cascade, ~1e-4 exact) that broadcasts csq_k
             onto every row for 512 cycles/group.  So PSUM holds the
             scores directly and no separate csq elementwise pass exists.
             fp16 input rounding perturbs scores by ~0.02 RMS which flips
             only boundary assignments (measured end-to-end rel err 0.014
             vs the 2e-2 gate).
  DVE:       per-half row-min of the psum scores (tensor_reduce x2), the
             combined min m, and the per-half exp biases B*m_h.
  ACT:       onehot halves: oh_h = Exp(-B*psum + B*m_h) read straight
             from PSUM per half (bias = that half's OWN min, so the exp
             fires as soon as the half is reduced - this keeps PSUM
             lifetimes short enough to double-buffer in 8 banks), written
             as fp8e4.  Also f_h = Exp(B*(m - m_h)): 1.0 for the half
             holding the global min, ~0 for the other.
  Pool:      rescales per-tile fp8 x_aug planes by f_h into two copies
             (the "loser" half's one-hot columns must shrink by f_h; we
             scale the mm2 x-side instead - chunks of k-half h use the
             f_h-scaled copy - which is 4x fewer elements).
  mm2 (PE):  partial_sums[k, d+1] = onehot^T @ x_aug per 16-tile group.
             x_aug is a 2-term unscaled fp8 split (xhi8 = fp8(x), xlo8 =
             fp8(x - xhi8), ~9-bit); MatmulPerfMode.DoubleRow processes
             two (onehot, x) tile pairs per instruction at 0.5 cyc/row.
             PSUM chunks evict via ACT copy -> SBUF -> DMA; host sums the
             8 group partials, divides by counts, keeps the old centroid
             where count==0.
"""

import numpy as np

import concourse.bass as bass
import concourse.mybir as mybir
import concourse.tile as tile
from concourse.bass_utils import run_bass_kernel_spmd
from concourse.vector_clock import ScopedClock

# ---------------------------------------------------------------------------
# Workaround: walrus rejects >1 sem wait on CTRL (drain/nop) instructions.
# Split the TileContext exit-drain's waits across one NOP per wait.
_MAXW = 1


def _patched_drain_and_barrier(self, tick_clock, wait_clock):
    nc = self.nc
    drain_inst = nc.sync.drain()
    wait_clock.add_sem_waits(
        drain_inst.ins, ScopedClock({None: tick_clock.global_clock})
    )
    si = drain_inst.ins.sync_info
    waits = list(si.on_wait) if si and si.on_wait else []
    if len(waits) > _MAXW:
        drain_inst.ins.sync_info = mybir.SyncInfo(
            on_wait=waits[:_MAXW], on_update=list(si.on_update or [])
        )
        rest = waits[_MAXW:]
        for i in range(0, len(rest), _MAXW):
            nop = nc.sync.nop()
            nop.ins.sync_info = mybir.SyncInfo(
                on_wait=rest[i : i + _MAXW], on_update=[]
            )
    nc.all_engine_barrier()
    popped = nc._tile_sem_poison_stack.pop()
    assert popped is self._sem_poison
    nc.clear_and_free_semaphores(list(self.sems.allocated().values()))
    nc.all_engine_barrier()


tile.TileContext._drain_and_barrier = _patched_drain_and_barrier

# This walrus build accepts only ONE sync wait per instruction, but Tile's
# scheduler emits several on phase joins.  Rewrite the BIR before compiling:
# excess waits move onto same-engine NOPs inserted just before the
# instruction (identical semantics: all waits still complete before it).
import json as _json

import concourse.bass2jax as _bass2jax

_orig_compile_bir = _bass2jax.compile_bir_kernel


def _split_waits_compile(bir_json, tmpdir, neff_name="file.neff"):
    j = _json.loads(bir_json)
    cnt = 0
    for f in j["functions"]:
        for bb in f["blocks"]:
            out = []
            for ins in bb["instructions"]:
                si = ins.get("sync_info")
                ow = (si or {}).get("on_wait") or []
                if len(ow) > 1:
                    for w in ow[:-1]:
                        cnt += 1
                        out.append(
                            {
                                "debug": ins.get("debug"),
                                "engine": ins["engine"],
                                "ins": [],
                                "outs": [],
                                "name": f"I-wsplit-{cnt}",
                                "opcode": "NoOp",
                                "sync_info": {"on_update": [], "on_wait": [w]},
                            }
                        )
                    si["on_wait"] = [ow[-1]]
                out.append(ins)
            bb["instructions"] = out
    return _orig_compile_bir(_json.dumps(j).encode(), tmpdir, neff_name=neff_name)


_bass2jax.compile_bir_kernel = _split_waits_compile
# ---------------------------------------------------------------------------

N_CORES = 8
P = 128
GROUP = 16
SOFTB = 256.0
QUADG = 4          # of the 4 512-col score groups, how many get csq via PE
F16 = mybir.dt.float16
F32 = mybir.dt.float32
F8 = mybir.dt.float8e4
ADD = mybir.AluOpType.add
MIN = mybir.AluOpType.min
SUB = mybir.AluOpType.subtract
MULT = mybir.AluOpType.mult
EXPF = mybir.ActivationFunctionType.Exp
DROW = mybir.MatmulPerfMode.DoubleRow
AXX = mybir.AxisListType.X

_KERNEL_CACHE = {}


def build_kernel(n_loc, k, d, group=GROUP, quadg=QUADG, ps1_bufs=3, ps2_bufs=2):
    ntiles = n_loc // P          # 128
    ndh = d // P                 # 2 contraction chunks
    nchunks = k // P             # 16 mm2 output chunks
    ngroups = ntiles // group    # 8
    npair = group // 2           # 8 tile-pairs per group
    assert nchunks == group
    daug = d + 1
    kh = k // 2
    nslots = ngroups             # one po slot per 16-tile group

    nc = bass.Bass()
    xt = nc.declare_dram_parameter("xt", [P, ntiles * ndh * P], F16, isOutput=False)
    xa8 = nc.declare_dram_parameter(
        "xa8", [P, (ntiles // 2) * 4, daug], F8, isOutput=False
    )
    cm = nc.declare_dram_parameter("cm", [P, ndh * k], F16, isOutput=False)
    csqb = nc.declare_dram_parameter("csqb", [P, k], F32, isOutput=False)
    csq16 = nc.declare_dram_parameter("csq16", [2, k], F16, isOutput=False)
    one16 = nc.declare_dram_parameter("one16", [2, P], F16, isOutput=False)
    po = nc.declare_dram_parameter("po", [k, nslots * daug], F32, isOutput=True)

    with tile.TileContext(nc) as tc:
        with (
            tc.tile_pool(name="consts", bufs=1) as consts,
            tc.tile_pool(name="xt", bufs=8) as xtp,
            tc.tile_pool(name="xa", bufs=npair + 2) as xap,
            tc.tile_pool(name="xc", bufs=2 * (npair + 1)) as xcp,
            tc.tile_pool(name="oh", bufs=2 * npair + 1) as ohp,
            tc.tile_pool(name="mx", bufs=8) as mxp,
            tc.tile_pool(name="st", bufs=4) as stp,
            tc.tile_pool(name="ps1", bufs=ps1_bufs, space="PSUM") as ps1,
            tc.tile_pool(name="ps2", bufs=ps2_bufs, space="PSUM") as ps2,
        ):
            c16t = consts.tile([2, k], F16, tag="c16", name="c16t")
            nc.sync.dma_start(out=c16t, in_=csq16[:, :])
            o16t = consts.tile([2, P], F16, tag="o16", name="o16t")
            nc.sync.dma_start(out=o16t, in_=one16[:, :])
            cmt = consts.tile([P, ndh * k], F16, tag="cm", name="cmt")

            # ordered so tile 0's earlier groups unblock first
            for lo in (0, k // 2):
                for j in range(ndh):
                    nc.sync.dma_start(
                        out=cmt[:, j * k + lo : j * k + lo + k // 2],
                        in_=cm[:, j * k + lo : j * k + lo + k // 2],
                    )
            if quadg < 4:
                csq = consts.tile([P, k], F32, tag="csq", name="csq")
                nc.sync.dma_start(out=csq, in_=csqb[:, :])

            def emit_mm2_chunk(grp, slot, c):
                # grp: list of (oh_pair [P,2k] f8, (xcp_h0, xcp_h1) [P,4,daug])
                half = c // (nchunks // 2)
                pc = ps2.tile([P, daug], F32, tag="ps2", name="pc")
                np_ = len(grp)
                for t, (o, xcs) in enumerate(grp):
                    o3 = o.rearrange("p (a b) -> p a b", a=2)
                    nc.tensor.matmul(
                        pc, o3[:, :, c * P : (c + 1) * P], xcs[half][:, 0:2, :],
                        start=(t == 0), stop=False, perf_mode=DROW,
                    )
                for t, (o, xcs) in enumerate(grp):
                    o3 = o.rearrange("p (a b) -> p a b", a=2)
                    nc.tensor.matmul(
                        pc, o3[:, :, c * P : (c + 1) * P], xcs[half][:, 2:4, :],
                        start=False, stop=(t == np_ - 1), perf_mode=DROW,
                    )
                st = stp.tile([P, daug], F32, tag="st", name="st")
                nc.scalar.copy(st, pc)
                nc.sync.dma_start(
                    out=po[c * P : (c + 1) * P, slot * daug : (slot + 1) * daug],
                    in_=st,
                )

            prev = None
            prev_gi = -1
            cur = []
            oht = None
            xat = None
            xcs = None
            for i in range(ntiles):
                xtt = xtp.tile([P, ndh * P], F16, tag="xt", name="xtt")
                nc.sync.dma_start(
                    out=xtt, in_=xt[:, i * ndh * P : (i + 1) * ndh * P]
                )
                if i % 2 == 0:
                    pi = i // 2
                    xat = xap.tile([P, 4, daug], F8, tag="xa", name="xat")
                    nc.sync.dma_start(
                        out=xat, in_=xa8[:, pi * 4 : (pi + 1) * 4, :]
                    )
                    oht = ohp.tile([P, 2 * k], F8, tag="oh", name="oht")
                    xc0 = xcp.tile([P, 4, daug], F8, tag="xc0", name="xc0")
                    xc1 = xcp.tile([P, 4, daug], F8, tag="xc1", name="xc1")
                    xcs = (xc0, xc1)

                phs = []
                for h in range(2):
                    ph = ps1.tile([P, 1024], F32, tag="ps1", name="ph")
                    phs.append(ph)
                    for q in range(2):
                        g = 2 * h + q
                        col = g * 512
                        quad = g < quadg
                        for j in range(ndh):
                            nc.tensor.matmul(
                                ph[:, q * 512 : (q + 1) * 512],
                                xtt[:, j * P : (j + 1) * P],
                                cmt[:, j * k + col : j * k + col + 512],
                                start=(j == 0),
                                stop=(j == ndh - 1 and not quad),
                            )
                        if quad:
                            # csq via one fp16 contraction-2 rank-2 matmul:
                            # ones[2,128]^T @ [csq_hi; csq_lo]
                            nc.tensor.matmul(
                                ph[:, q * 512 : (q + 1) * 512],
                                o16t,
                                c16t[:, col : col + 512],
                                start=False, stop=True,
                            )

                half = i % 2
                mh = []
                bh = []
                for h in range(2):
                    # csq for non-quad groups of this half: in-place DVE add
                    for g in range(2 * h, 2 * h + 2):
                        if g >= quadg:
                            q = g - 2 * h
                            nc.vector.tensor_tensor(
                                phs[h][:, q * 512 : (q + 1) * 512],
                                phs[h][:, q * 512 : (q + 1) * 512],
                                csq[:, g * 512 : (g + 1) * 512],
                                op=ADD,
                            )
                    m_h = mxp.tile([P, 1], F32, tag=f"m{h}", name=f"m{h}")
                    nc.vector.tensor_reduce(m_h, phs[h], axis=AXX, op=MIN)
                    b_h = mxp.tile([P, 1], F32, tag=f"b{h}", name=f"b{h}")
                    nc.vector.tensor_scalar_mul(b_h, m_h, SOFTB)
                    # exp of this half as soon as its own min is ready
                    nc.scalar.activation(
                        oht[:, half * k + h * kh : half * k + (h + 1) * kh],
                        phs[h], EXPF, bias=b_h, scale=-SOFTB,
                    )
                    mh.append(m_h)
                    bh.append(b_h)

                mm = mxp.tile([P, 1], F32, tag="mm", name="mm")
                nc.vector.tensor_scalar(
                    out=mm, in0=mh[0], scalar1=mh[1], scalar2=None, op0=MIN
                )
                # f_h = exp(B*(m - m_h)); the winner half gets exactly 1.0
                a2 = mxp.tile([P, 2], F32, tag="a2", name="a2")
                for h in range(2):
                    nc.gpsimd.tensor_scalar(
                        out=a2[:, h : h + 1], in0=mm, scalar1=mh[h],
                        scalar2=SOFTB, op0=SUB, op1=MULT,
                    )
                f2 = mxp.tile([P, 2], F32, tag="f2", name="f2")
                nc.scalar.activation(f2, a2, EXPF, bias=0.0, scale=1.0)
                # scaled x_aug planes for this tile (planes half::2 are its
                # hi/lo), one copy per k-half
                for h in range(2):
                    nc.gpsimd.tensor_scalar_mul(
                        xcs[h][:, half : half + 3 : 2, :],
                        xat[:, half : half + 3 : 2, :],
                        f2[:, h : h + 1],
                    )
                if half == 1:
                    cur.append((oht, xcs))

                # interleave the previous group's mm2: one chunk per tile
                if prev is not None:
                    emit_mm2_chunk(prev, prev_gi, i % group)

                if len(cur) == npair:
                    prev = cur
                    prev_gi += 1
                    cur = []

            # tail: last group's mm2
            if prev is not None:
                for c in range(nchunks):
                    emit_mm2_chunk(prev, prev_gi, c)
    return nc


def _prep_inputs(x, C):
    import ml_dtypes

    F8NP = ml_dtypes.float8_e4m3
    n, d = x.shape
    k = C.shape[0]
    n_loc = n // N_CORES
    ntiles = n_loc // P
    npairs = ntiles // 2
    ndh = d // P
    daug = d + 1

    CmT = np.ascontiguousarray((-2.0 * C.astype(np.float64)).T.astype(np.float16))
    cm = np.concatenate([CmT[j * P : (j + 1) * P, :] for j in range(ndh)], axis=1)
    c_sq = np.sum(C.astype(np.float64) ** 2, axis=1).astype(np.float32)
    csqb = np.ascontiguousarray(np.broadcast_to(c_sq, (P, k)))
    # 2-term fp16 cascade of csq for the contraction-2 rank-2 matmul
    c0 = c_sq.astype(np.float64).astype(np.float16)
    c1 = (c_sq.astype(np.float64) - c0.astype(np.float64)).astype(np.float16)
    csq16 = np.ascontiguousarray(np.stack([c0, c1], axis=0))  # [2,k]
    one16 = np.ones((2, P), np.float16)

    x16 = x.astype(np.float16)
    xhi8 = x.astype(F8NP)
    xlo8 = (x - xhi8.astype(np.float32)).astype(F8NP)
    in_maps = []
    for c in range(N_CORES):
        sl = slice(c * n_loc, (c + 1) * n_loc)
        xs = x16[sl]                                         # [n_loc, d]
        t = xs.reshape(ntiles, P, ndh, P)                    # [i, nrow, jchunk, didx]
        xt = np.ascontiguousarray(
            t.transpose(3, 0, 2, 1).reshape(P, ntiles * ndh * P)
        )                                                    # [didx | i, j, nrow]
        # fp8 pair planes [hi(2p), hi(2p+1), lo(2p), lo(2p+1)], each [P, daug]
        hi = np.concatenate(
            [xhi8[sl].reshape(npairs, 2, P, d),
             np.ones((npairs, 2, P, 1), F8NP)], axis=3
        )
        lo = np.concatenate(
            [xlo8[sl].reshape(npairs, 2, P, d),
             np.zeros((npairs, 2, P, 1), F8NP)], axis=3
        )
        planes = np.concatenate([hi, lo], axis=1)            # [pi, 4, P, daug]
        xa8 = np.ascontiguousarray(
            planes.transpose(2, 0, 1, 3).reshape(P, npairs * 4, daug)
        )
        in_maps.append(
            {"xt": xt, "xa8": xa8, "cm": cm, "csqb": csqb, "csq16": csq16,
             "one16": one16}
        )
    return in_maps


def kernel(x, centroids, _trace=False):
    x = np.asarray(x, dtype=np.float32)
    C = np.asarray(centroids, dtype=np.float32)
    n, d = x.shape
    k = C.shape[0]
    n_loc = n // N_CORES
    nslots = (n_loc // P) // GROUP       # 8
    daug = d + 1

    key = (n_loc, k, d)
    if key not in _KERNEL_CACHE:
        _KERNEL_CACHE[key] = build_kernel(n_loc, k, d)
    nc = _KERNEL_CACHE[key]

    in_maps = _prep_inputs(x, C)
    res = run_bass_kernel_spmd(
        nc, in_maps, core_ids=list(range(N_CORES)), trace=_trace
    )

    total = np.zeros((k, daug), np.float64)
    for c in range(N_CORES):
        po = res.results[c]["po"].astype(np.float64)
        total += po.reshape(k, nslots, daug).sum(axis=1)
    sums = total[:, :d]
    counts = total[:, d]
    means = (sums / np.maximum(counts, 1.0)[:, None]).astype(np.float32)
    out = np.where(counts[:, None] > 0.5, means, C)
    if _trace:
        kernel._last_result = res
    return out.astype(np.float32)


# revision 38
# speedup vs baseline: 1.0090x; 1.0090x over previous
"""K-means step kernel for Trainium2 (8 NeuronCores, data-parallel over n).

scores[n,k] = ||c_k||^2 - 2 x_n.c_k ; assign = argmin_k ; new centroids =
segment-mean.  Strategy per core (n_loc = n/8 rows, 128 tiles of 128 rows):

  mm1 (PE):  psum = x16 @ (-2C)16^T in one fp16 pass (fp32 PSUM accumulate)
             with csq added IN PSUM: each 512-col group gets one extra
             fp16 contraction-2 matmul ones[2,128]^T @ [csq_hi; csq_lo]
             (2-term fp16 cascade, ~1e-4 exact) that broadcasts csq_k
             onto every row for 512 cycles/group.  So PSUM holds the
             scores directly and no separate csq elementwise pass exists.
             fp16 input rounding perturbs scores by ~0.02 RMS which flips
             only boundary assignments (measured end-to-end rel err 0.014
             vs the 2e-2 gate).
  DVE:       per-half row-min of the psum scores (tensor_reduce x2), the
             combined min m, and the per-half exp biases B*m_h.
  ACT:       onehot halves: oh_h = Exp(-B*psum + B*m_h) read straight
             from PSUM per half (bias = that half's OWN min, so the exp
             fires as soon as the half is reduced - this keeps PSUM
             lifetimes short enough to double-buffer in 8 banks), written
             as fp8e4.  Also f_h = Exp(B*(m - m_h)): 1.0 for the half
             holding the global min, ~0 for the other.
  Pool:      rescales per-tile fp8 x_aug planes by f_h into two copies
             (the "loser" half's one-hot columns must shrink by f_h; we
             scale the mm2 x-side instead - chunks of k-half h use the
             f_h-scaled copy - which is 4x fewer elements).
  mm2 (PE):  partial_sums[k, d+1] = onehot^T @ x_aug per 16-tile group.
             x_aug is a 2-term unscaled fp8 split (xhi8 = fp8(x), xlo8 =
             fp8(x - xhi8), ~9-bit); MatmulPerfMode.DoubleRow processes
             two (onehot, x) tile pairs per instruction at 0.5 cyc/row.
             PSUM chunks evict via ACT copy -> SBUF -> DMA; host sums the
             8 group partials, divides by counts, keeps the old centroid
             where count==0.
"""

import numpy as np

import concourse.bass as bass
import concourse.mybir as mybir
import concourse.tile as tile
from concourse.bass_utils import run_bass_kernel_spmd
from concourse.vector_clock import ScopedClock

# ---------------------------------------------------------------------------
# Workaround: walrus rejects >1 sem wait on CTRL (drain/nop) instructions.
# Split the TileContext exit-drain's waits across one NOP per wait.
_MAXW = 1


def _patched_drain_and_barrier(self, tick_clock, wait_clock):
    nc = self.nc
    drain_inst = nc.sync.drain()
    wait_clock.add_sem_waits(
        drain_inst.ins, ScopedClock({None: tick_clock.global_clock})
    )
    si = drain_inst.ins.sync_info
    waits = list(si.on_wait) if si and si.on_wait else []
    if len(waits) > _MAXW:
        drain_inst.ins.sync_info = mybir.SyncInfo(
            on_wait=waits[:_MAXW], on_update=list(si.on_update or [])
        )
        rest = waits[_MAXW:]
        for i in range(0, len(rest), _MAXW):
            nop = nc.sync.nop()
            nop.ins.sync_info = mybir.SyncInfo(
                on_wait=rest[i : i + _MAXW], on_update=[]
            )
    nc.all_engine_barrier()
    popped = nc._tile_sem_poison_stack.pop()
    assert popped is self._sem_poison
    nc.clear_and_free_semaphores(list(self.sems.allocated().values()))
    nc.all_engine_barrier()


tile.TileContext._drain_and_barrier = _patched_drain_and_barrier

# This walrus build accepts only ONE sync wait per instruction, but Tile's
# scheduler emits several on phase joins.  Rewrite the BIR before compiling:
# excess waits move onto same-engine NOPs inserted just before the
# instruction (identical semantics: all waits still complete before it).
import json as _json

import concourse.bass2jax as _bass2jax

_orig_compile_bir = _bass2jax.compile_bir_kernel


def _split_waits_compile(bir_json, tmpdir, neff_name="file.neff"):
    j = _json.loads(bir_json)
    cnt = 0
    for f in j["functions"]:
        for bb in f["blocks"]:
            out = []
            for ins in bb["instructions"]:
                si = ins.get("sync_info")
                ow = (si or {}).get("on_wait") or []
                if len(ow) > 1:
                    for w in ow[:-1]:
                        cnt += 1
                        out.append(
                            {
                                "debug": ins.get("debug"),
                                "engine": ins["engine"],
                                "ins": [],
                                "outs": [],
                                "name": f"I-wsplit-{cnt}",
                                "opcode": "NoOp",
                                "sync_info": {"on_update": [], "on_wait": [w]},
                            }
                        )
                    si["on_wait"] = [ow[-1]]
                out.append(ins)
            bb["instructions"] = out
    return _orig_compile_bir(_json.dumps(j).encode(), tmpdir, neff_name=neff_name)


_bass2jax.compile_bir_kernel = _split_waits_compile
# ---------------------------------------------------------------------------

N_CORES = 8
P = 128
GROUP = 8
SOFTB = 256.0
QUADG = 4          # of the 4 512-col score groups, how many get csq via PE
F16 = mybir.dt.float16
F32 = mybir.dt.float32
F8 = mybir.dt.float8e4
ADD = mybir.AluOpType.add
MIN = mybir.AluOpType.min
SUB = mybir.AluOpType.subtract
MULT = mybir.AluOpType.mult
EXPF = mybir.ActivationFunctionType.Exp
DROW = mybir.MatmulPerfMode.DoubleRow
AXX = mybir.AxisListType.X

_KERNEL_CACHE = {}


def build_kernel(n_loc, k, d, group=GROUP, quadg=QUADG, ps1_bufs=3, ps2_bufs=2):
    ntiles = n_loc // P          # 128
    ndh = d // P                 # 2 contraction chunks
    nchunks = k // P             # 16 mm2 output chunks
    ngroups = ntiles // group    # 8
    npair = group // 2           # tile-pairs per group
    assert nchunks % group == 0
    cpt = nchunks // group       # chunks emitted per tile
    daug = d + 1
    kh = k // 2
    nslots = ngroups             # one po slot per 16-tile group

    nc = bass.Bass()
    xt = nc.declare_dram_parameter("xt", [P, ntiles * ndh * P], F16, isOutput=False)
    xa8 = nc.declare_dram_parameter(
        "xa8", [P, (ntiles // 2) * 4, daug], F8, isOutput=False
    )
    cm = nc.declare_dram_parameter("cm", [P, ndh * k], F16, isOutput=False)
    csqb = nc.declare_dram_parameter("csqb", [P, k], F32, isOutput=False)
    csq16 = nc.declare_dram_parameter("csq16", [2, k], F16, isOutput=False)
    one16 = nc.declare_dram_parameter("one16", [2, P], F16, isOutput=False)
    po = nc.declare_dram_parameter("po", [k, nslots * daug], F32, isOutput=True)

    with tile.TileContext(nc) as tc:
        with (
            tc.tile_pool(name="consts", bufs=1) as consts,
            tc.tile_pool(name="xt", bufs=8) as xtp,
            tc.tile_pool(name="xa", bufs=npair + 2) as xap,
            tc.tile_pool(name="xc", bufs=2 * (npair + 1)) as xcp,
            tc.tile_pool(name="oh", bufs=2 * npair + 1) as ohp,
            tc.tile_pool(name="mx", bufs=8) as mxp,
            tc.tile_pool(name="st", bufs=4) as stp,
            tc.tile_pool(name="ps1", bufs=ps1_bufs, space="PSUM") as ps1,
            tc.tile_pool(name="ps2", bufs=ps2_bufs, space="PSUM") as ps2,
        ):
            # startup ordering: everything tile 0 group 0 needs goes first
            cmt = consts.tile([P, ndh * k], F16, tag="cm", name="cmt")
            c16t = consts.tile([2, k], F16, tag="c16", name="c16t")
            o16t = consts.tile([2, P], F16, tag="o16", name="o16t")
            nc.sync.dma_start(out=cmt[:, 0 : k // 2], in_=cm[:, 0 : k // 2])
            nc.sync.dma_start(
                out=cmt[:, k : k + k // 2], in_=cm[:, k : k + k // 2]
            )
            nc.sync.dma_start(out=o16t, in_=one16[:, :])
            nc.sync.dma_start(out=c16t[:, 0 : k // 2], in_=csq16[:, 0 : k // 2])
            nc.sync.dma_start(out=c16t[:, k // 2 :], in_=csq16[:, k // 2 :])
            for j in range(ndh):
                nc.sync.dma_start(
                    out=cmt[:, j * k + k // 2 : (j + 1) * k],
                    in_=cm[:, j * k + k // 2 : (j + 1) * k],
                )
            if quadg < 4:
                csq = consts.tile([P, k], F32, tag="csq", name="csq")
                nc.sync.dma_start(out=csq, in_=csqb[:, :])

            def emit_mm2_chunk(grp, slot, c):
                # grp: list of (oh_pair [P,2k] f8, (xcp_h0, xcp_h1) [P,4,daug])
                half = c // (nchunks // 2)
                pc = ps2.tile([P, daug], F32, tag="ps2", name="pc")
                np_ = len(grp)
                for t, (o, xcs) in enumerate(grp):
                    o3 = o.rearrange("p (a b) -> p a b", a=2)
                    nc.tensor.matmul(
                        pc, o3[:, :, c * P : (c + 1) * P], xcs[half][:, 0:2, :],
                        start=(t == 0), stop=False, perf_mode=DROW,
                    )
                for t, (o, xcs) in enumerate(grp):
                    o3 = o.rearrange("p (a b) -> p a b", a=2)
                    nc.tensor.matmul(
                        pc, o3[:, :, c * P : (c + 1) * P], xcs[half][:, 2:4, :],
                        start=False, stop=(t == np_ - 1), perf_mode=DROW,
                    )
                st = stp.tile([P, daug], F32, tag="st", name="st")
                nc.scalar.copy(st, pc)
                nc.sync.dma_start(
                    out=po[c * P : (c + 1) * P, slot * daug : (slot + 1) * daug],
                    in_=st,
                )

            prev = None
            prev_gi = -1
            cur = []
            oht = None
            xat = None
            xcs = None
            for i in range(ntiles):
                xtt = xtp.tile([P, ndh * P], F16, tag="xt", name="xtt")
                nc.sync.dma_start(
                    out=xtt, in_=xt[:, i * ndh * P : (i + 1) * ndh * P]
                )
                if i % 2 == 0:
                    pi = i // 2
                    xat = xap.tile([P, 4, daug], F8, tag="xa", name="xat")
                    nc.sync.dma_start(
                        out=xat, in_=xa8[:, pi * 4 : (pi + 1) * 4, :]
                    )
                    oht = ohp.tile([P, 2 * k], F8, tag="oh", name="oht")
                    xc0 = xcp.tile([P, 4, daug], F8, tag="xc0", name="xc0")
                    xc1 = xcp.tile([P, 4, daug], F8, tag="xc1", name="xc1")
                    xcs = (xc0, xc1)

                phs = []
                for h in range(2):
                    ph = ps1.tile([P, 1024], F32, tag="ps1", name="ph")
                    phs.append(ph)
                    for q in range(2):
                        g = 2 * h + q
                        col = g * 512
                        quad = g < quadg
                        for j in range(ndh):
                            nc.tensor.matmul(
                                ph[:, q * 512 : (q + 1) * 512],
                                xtt[:, j * P : (j + 1) * P],
                                cmt[:, j * k + col : j * k + col + 512],
                                start=(j == 0),
                                stop=(j == ndh - 1 and not quad),
                            )
                        if quad:
                            # csq via one fp16 contraction-2 rank-2 matmul:
                            # ones[2,128]^T @ [csq_hi; csq_lo]
                            nc.tensor.matmul(
                                ph[:, q * 512 : (q + 1) * 512],
                                o16t,
                                c16t[:, col : col + 512],
                                start=False, stop=True,
                            )

                half = i % 2
                mh = []
                bh = []
                for h in range(2):
                    # csq for non-quad groups of this half: in-place DVE add
                    for g in range(2 * h, 2 * h + 2):
                        if g >= quadg:
                            q = g - 2 * h
                            nc.vector.tensor_tensor(
                                phs[h][:, q * 512 : (q + 1) * 512],
                                phs[h][:, q * 512 : (q + 1) * 512],
                                csq[:, g * 512 : (g + 1) * 512],
                                op=ADD,
                            )
                    m_h = mxp.tile([P, 1], F32, tag=f"m{h}", name=f"m{h}")
                    nc.vector.tensor_reduce(m_h, phs[h], axis=AXX, op=MIN)
                    b_h = mxp.tile([P, 1], F32, tag=f"b{h}", name=f"b{h}")
                    nc.vector.tensor_scalar_mul(b_h, m_h, SOFTB)
                    # exp of this half as soon as its own min is ready
                    nc.scalar.activation(
                        oht[:, half * k + h * kh : half * k + (h + 1) * kh],
                        phs[h], EXPF, bias=b_h, scale=-SOFTB,
                    )
                    mh.append(m_h)
                    bh.append(b_h)

                mm = mxp.tile([P, 1], F32, tag="mm", name="mm")
                nc.vector.tensor_scalar(
                    out=mm, in0=mh[0], scalar1=mh[1], scalar2=None, op0=MIN
                )
                # f_h = exp(B*(m - m_h)); the winner half gets exactly 1.0
                a2 = mxp.tile([P, 2], F32, tag="a2", name="a2")
                for h in range(2):
                    nc.gpsimd.tensor_scalar(
                        out=a2[:, h : h + 1], in0=mm, scalar1=mh[h],
                        scalar2=SOFTB, op0=SUB, op1=MULT,
                    )
                f2 = mxp.tile([P, 2], F32, tag="f2", name="f2")
                nc.scalar.activation(f2, a2, EXPF, bias=0.0, scale=1.0)
                # scaled x_aug planes for this tile (planes half::2 are its
                # hi/lo), one copy per k-half
                for h in range(2):
                    nc.gpsimd.tensor_scalar_mul(
                        xcs[h][:, half : half + 3 : 2, :],
                        xat[:, half : half + 3 : 2, :],
                        f2[:, h : h + 1],
                    )
                if half == 1:
                    cur.append((oht, xcs))

                # interleave the previous group's mm2 chunks across this group
                if prev is not None:
                    for cc in range(cpt):
                        emit_mm2_chunk(prev, prev_gi, (i % group) * cpt + cc)

                if len(cur) == npair:
                    prev = cur
                    prev_gi += 1
                    cur = []

            # tail: last group's mm2
            if prev is not None:
                for c in range(nchunks):
                    emit_mm2_chunk(prev, prev_gi, c)
    return nc


def _prep_inputs(x, C):
    import ml_dtypes

    F8NP = ml_dtypes.float8_e4m3
    n, d = x.shape
    k = C.shape[0]
    n_loc = n // N_CORES
    ntiles = n_loc // P
    npairs = ntiles // 2
    ndh = d // P
    daug = d + 1

    CmT = np.ascontiguousarray((-2.0 * C.astype(np.float64)).T.astype(np.float16))
    cm = np.concatenate([CmT[j * P : (j + 1) * P, :] for j in range(ndh)], axis=1)
    c_sq = np.sum(C.astype(np.float64) ** 2, axis=1).astype(np.float32)
    csqb = np.ascontiguousarray(np.broadcast_to(c_sq, (P, k)))
    # 2-term fp16 cascade of csq for the contraction-2 rank-2 matmul
    c0 = c_sq.astype(np.float64).astype(np.float16)
    c1 = (c_sq.astype(np.float64) - c0.astype(np.float64)).astype(np.float16)
    csq16 = np.ascontiguousarray(np.stack([c0, c1], axis=0))  # [2,k]
    one16 = np.ones((2, P), np.float16)

    x16 = x.astype(np.float16)
    xhi8 = x.astype(F8NP)
    xlo8 = (x - xhi8.astype(np.float32)).astype(F8NP)
    in_maps = []
    for c in range(N_CORES):
        sl = slice(c * n_loc, (c + 1) * n_loc)
        xs = x16[sl]                                         # [n_loc, d]
        t = xs.reshape(ntiles, P, ndh, P)                    # [i, nrow, jchunk, didx]
        xt = np.ascontiguousarray(
            t.transpose(3, 0, 2, 1).reshape(P, ntiles * ndh * P)
        )                                                    # [didx | i, j, nrow]
        # fp8 pair planes [hi(2p), hi(2p+1), lo(2p), lo(2p+1)], each [P, daug]
        hi = np.concatenate(
            [xhi8[sl].reshape(npairs, 2, P, d),
             np.ones((npairs, 2, P, 1), F8NP)], axis=3
        )
        lo = np.concatenate(
            [xlo8[sl].reshape(npairs, 2, P, d),
             np.zeros((npairs, 2, P, 1), F8NP)], axis=3
        )
        planes = np.concatenate([hi, lo], axis=1)            # [pi, 4, P, daug]
        xa8 = np.ascontiguousarray(
            planes.transpose(2, 0, 1, 3).reshape(P, npairs * 4, daug)
        )
        in_maps.append(
            {"xt": xt, "xa8": xa8, "cm": cm, "csqb": csqb, "csq16": csq16,
             "one16": one16}
        )
    return in_maps


def kernel(x, centroids, _trace=False):
    x = np.asarray(x, dtype=np.float32)
    C = np.asarray(centroids, dtype=np.float32)
    n, d = x.shape
    k = C.shape[0]
    n_loc = n // N_CORES
    nslots = (n_loc // P) // GROUP       # 8
    daug = d + 1

    key = (n_loc, k, d)
    if key not in _KERNEL_CACHE:
        _KERNEL_CACHE[key] = build_kernel(n_loc, k, d)
    nc = _KERNEL_CACHE[key]

    in_maps = _prep_inputs(x, C)
    res = run_bass_kernel_spmd(
        nc, in_maps, core_ids=list(range(N_CORES)), trace=_trace
    )

    total = np.zeros((k, daug), np.float64)
    for c in range(N_CORES):
        po = res.results[c]["po"].astype(np.float64)
        total += po.reshape(k, nslots, daug).sum(axis=1)
    sums = total[:, :d]
    counts = total[:, d]
    means = (sums / np.maximum(counts, 1.0)[:, None]).astype(np.float32)
    out = np.where(counts[:, None] > 0.5, means, C)
    if _trace:
        kernel._last_result = res
    return out.astype(np.float32)
